# revision 27
# baseline (speedup 1.0000x reference)
"""Additive attention (Bahdanau) Trainium2 kernel, 8-core data parallel.

out = softmax_k(mask(sum_h w_v[h] * tanh(qf[q,h] + kf[k,h]))) @ V
with qf = q@Wq, kf = k@Wk.

Instead of materializing tanh over the [nq, nk, H] feature tensor (16.8M
ACT elements per core -> ~110us floor), tanh is expanded in a sine series

    tanh(s) ~ sum_m beta_m sin(m*w0*s),   s = qf + kf

and each sin(m*w0*(a+b)) = sin(m*w0*a)cos(m*w0*b) + cos(m*w0*a)sin(m*w0*b)
is SEPARABLE: the score reduction over h becomes 2 matmuls per harmonic on
the (otherwise idle) PE.  Only the small projected tensors qf [64,256] and
kf [512,256] ever pass through trig:

  ACT:  sin/cos fundamentals (args kept in [-pi,pi] via a clamp fused into
        the PSUM evacuation) + squares, then the softmax exps
        (2 table loads total: trig_and_small, exp_and_others)
  DVE:  harmonic ladder in bf16 (double-angle + angle-addition products;
        scale factors from halved products folded into beta / TS imms)
  PE :  projections, 32 accumulating score matmuls per batch, attn @ V

Harmonics {1,2,3,4,5,6,8,10}, period 2P = 21 (w0 = 2pi/21), clamp 5.2.
Numpy-simulated end-to-end rel err vs the exact reference: 5.0e-3
(gate 2e-2).
"""

import os
from contextlib import ExitStack

import ml_dtypes
import numpy as np

import concourse.bacc as bacc
import concourse.bass as bass
import concourse.mybir as mybir
import concourse.tile as tile
from concourse.bass_utils import run_bass_kernel_spmd

F32 = mybir.dt.float32
BF16 = mybir.dt.bfloat16
I32 = mybir.dt.int32
AF = mybir.ActivationFunctionType
ALU = mybir.AluOpType

B, NQ, NK, QS, KS, H, VD = 16, 64, 512, 256, 256, 256, 256
NCORES = 8
BPC = B // NCORES  # batches per core
MASK_NEG = -30.0

PI = float(np.pi)
W0 = 2.0 * PI / 21.0       # fundamental frequency (P = 10.5)
CLAMP = 5.2                # |qf|,|kf| clamp so all ACT sin args stay in range
HSET = [1, 2, 3, 4, 5, 6, 8, 10]
# ridge fit of tanh on [-10.5, 10.5], gaussian(sigma=sqrt(2)) + 1e-3 floor
# weighting, with the ladder's scale factors folded in: s3,c3,s5,c5 stored
# at -1/4 scale (x16 on 3,5), s4 at 1/2 (x2), s8 at 1/4 (x4), s6,s10 at
# 1/32 (x32)
BETA_F = [1.24833822, -0.03631956, 5.91820800, -0.16552149,
          2.89872992, -0.07516896, 0.14800902, 0.42580896]


def _build():
    nc = bacc.Bacc()
    q_d = nc.declare_dram_parameter("queries", [BPC, NQ, QS], F32, isOutput=False)
    k_d = nc.declare_dram_parameter("keys", [BPC, NK, KS], F32, isOutput=False)
    v_d = nc.declare_dram_parameter("values", [BPC, NK, VD], F32, isOutput=False)
    vl_d = nc.declare_dram_parameter("valid_lens", [BPC, 1], I32, isOutput=False)
    wq_d = nc.declare_dram_parameter("W_q", [QS, H], F32, isOutput=False)
    wk_d = nc.declare_dram_parameter("W_k", [KS, H], F32, isOutput=False)
    wv_d = nc.declare_dram_parameter("w_v", [H], F32, isOutput=False)
    out_d = nc.declare_dram_parameter("out", [BPC, NQ, VD], F32, isOutput=True)

    identb_d = nc.inline_tensor(
        np.eye(128).astype(ml_dtypes.bfloat16), name="identb_c"
    )
    krow_d = nc.inline_tensor(np.arange(NK, dtype=np.float32), name="krow_c")

    with ExitStack() as ctx:
        tc = ctx.enter_context(tile.TileContext(nc))
        consts = ctx.enter_context(tc.tile_pool(name="consts", bufs=1))
        setup = ctx.enter_context(tc.tile_pool(name="setup", bufs=2))
        lad = ctx.enter_context(tc.tile_pool(name="lad", bufs=2))
        qlad = ctx.enter_context(tc.tile_pool(name="qlad", bufs=1))
        sm = ctx.enter_context(tc.tile_pool(name="sm", bufs=1))
        outp = ctx.enter_context(tc.tile_pool(name="outp", bufs=2))
        ps_sc = ctx.enter_context(tc.tile_pool(name="ps_sc", bufs=2, space="PSUM"))
        ps_misc = ctx.enter_context(tc.tile_pool(name="ps_misc", bufs=2, space="PSUM"))
        ps_out = ctx.enter_context(tc.tile_pool(name="ps_out", bufs=2, space="PSUM"))

        # ---------------- loads (critical first, two queues) ----------------
        k_sb0 = setup.tile([128, 4, KS], F32, tag="k_sb0", bufs=1)
        k0_view = k_d[0].rearrange("(kb p) d -> p kb d", p=128)
        identb = consts.tile([128, 128], BF16)
        nc.sync.dma_start(out=k_sb0[:, 0], in_=k0_view[:, 0])
        nc.gpsimd.dma_start(out=identb, in_=identb_d[:, :])
        nc.gpsimd.dma_start(out=k_sb0[:, 1], in_=k0_view[:, 1])
        nc.sync.dma_start(out=k_sb0[:, 2], in_=k0_view[:, 2])
        nc.gpsimd.dma_start(out=k_sb0[:, 3], in_=k0_view[:, 3])
        wk_sb = setup.tile([128, 2, H], F32, tag="wk_f", bufs=1)
        nc.gpsimd.dma_start(out=wk_sb, in_=wk_d.rearrange("(kt p) m -> p kt m", p=128))
        wq_sb = setup.tile([128, 2, H], F32, tag="wq_f", bufs=1)
        nc.sync.dma_start(out=wq_sb, in_=wq_d.rearrange("(kt p) m -> p kt m", p=128))
        q_sb0 = setup.tile([NQ, QS], F32, tag="q_sb0", bufs=1)
        nc.sync.dma_start(out=q_sb0, in_=q_d[0])
        q_sb1 = setup.tile([NQ, QS], F32, tag="q_sb1", bufs=1)
        nc.sync.dma_start(out=q_sb1, in_=q_d[1])
        wv_col = consts.tile([128, 2], F32)
        nc.gpsimd.dma_start(out=wv_col, in_=wv_d.rearrange("(t p) -> p t", p=128))
        k_sb1 = setup.tile([128, 4, KS], F32, tag="k_sb1", bufs=1)
        k1_view = k_d[1].rearrange("(kb p) d -> p kb d", p=128)
        for kb in range(4):
            eng = nc.sync if kb % 2 == 0 else nc.gpsimd
            eng.dma_start(out=k_sb1[:, kb], in_=k1_view[:, kb])
        krow = consts.tile([128, NK], F32)
        nc.sync.dma_start(out=krow, in_=krow_d[None, :].partition_broadcast(128))
        v_sbs, valid_sbs = [], []
        for b in range(BPC):
            v_sb = setup.tile([128, 4, VD], F32, tag=f"v_sb{b}", name=f"v_sb{b}", bufs=1)
            nc.gpsimd.dma_start(
                out=v_sb, in_=v_d[b].rearrange("(kb p) d -> p kb d", p=128)
            )
            v_sbs.append(v_sb)
            valid_sb = setup.tile([128, 1], I32, tag=f"valid{b}", name=f"valid{b}")
            nc.gpsimd.dma_start(
                out=valid_sb, in_=vl_d[b : b + 1, :].partition_broadcast(128)
            )
            valid_sbs.append(valid_sb)

        k_sbs = [k_sb0, k_sb1]
        q_sbs = [q_sb0, q_sb1]

        # ACT bias constants (per-partition APs)
        halfpi = consts.tile([128, 1], F32)
        nc.gpsimd.memset(halfpi, PI / 2)

        # weights to bf16 (first on the DVE stream); valid_lens to f32
        wq_bf = consts.tile([128, 2, H], BF16)
        wk_bf = consts.tile([128, 2, H], BF16)
        nc.vector.tensor_copy(out=wk_bf, in_=wk_sb)
        nc.vector.tensor_copy(out=wq_bf, in_=wq_sb)
        valid_fs = []
        for b in range(BPC):
            valid_f = setup.tile([128, 1], F32, tag="validf", name=f"vf{b}")
            nc.vector.tensor_copy(out=valid_f, in_=valid_sbs[b])
            valid_fs.append(valid_f)

        # ---------------- transposes + projections ----------------
        def transpose_project_k(b):
            """per-block bf16 cast -> bf16 PE transposes -> sliced projection."""
            k_bf = setup.tile([128, 4, KS], BF16, tag="k_bf", name=f"k_bf{b}")
            kT_bf = setup.tile([128, 2, NK], BF16, tag="kT", name=f"kT{b}")
            for kb in range(4):
                nc.vector.tensor_copy(out=k_bf[:, kb], in_=k_sbs[b][:, kb])
                for kt in range(2):
                    pst = ps_misc.tile([128, 1024], BF16, tag="ps_miscb", name="pst_k")
                    nc.tensor.transpose(
                        pst[:, 0:128], k_bf[:, kb, kt * 128 : (kt + 1) * 128], identb
                    )
                    nc.vector.tensor_copy(
                        out=kT_bf[:, kt, kb * 128 : (kb + 1) * 128], in_=pst[:, 0:128]
                    )
            kfT = setup.tile([128, 2, NK], F32, tag="kfT", name=f"kfT{b}")
            psps = []
            for mt in range(2):
                psps.append(ps_misc.tile([128, 512], F32, tag="ps_misc",
                                         name=f"psp_k{mt}"))
            # per-block slices so each projection only waits on its own block
            for mt in range(2):
                for kb in range(4):
                    sl = slice(kb * 128, (kb + 1) * 128)
                    for kt in range(2):
                        nc.tensor.matmul(
                            psps[mt][:, sl],
                            lhsT=wk_bf[:, kt, mt * 128 : (mt + 1) * 128],
                            rhs=kT_bf[:, kt, sl],
                            start=(kt == 0),
                            stop=(kt == 1),
                        )
            for mt in range(2):
                # evacuate + clamp to +-CLAMP in one DVE op
                nc.vector.tensor_scalar(
                    out=kfT[:, mt], in0=psps[mt], scalar1=CLAMP, scalar2=-CLAMP,
                    op0=ALU.min, op1=ALU.max,
                )
            return kfT

        def transpose_project_q():
            """Both batches' q -> combined clamped qfT [128, 2ht, 2b, NQ]."""
            qfT = qlad.tile([128, 2, BPC, NQ], F32, tag="qfT", name="qfT")
            qT_bfs = []
            for b in range(BPC):
                q_bf = setup.tile([NQ, QS], BF16, tag="q_bf", name=f"q_bf{b}")
                nc.vector.tensor_copy(out=q_bf, in_=q_sbs[b])
                qT_bf = setup.tile([128, 2, NQ], BF16, tag="qT", name=f"qT{b}")
                for kt in range(2):
                    pst = ps_misc.tile([128, 1024], BF16, tag="ps_miscb", name="pst_q")
                    nc.tensor.transpose(
                        pst[:, 0:NQ],
                        q_bf[:, kt * 128 : (kt + 1) * 128],
                        identb[0:NQ, 0:NQ],
                    )
                    nc.vector.tensor_copy(out=qT_bf[:, kt, :], in_=pst[:, 0:NQ])
                qT_bfs.append(qT_bf)
            for b in range(BPC):
                for mt in range(2):
                    psp = ps_misc.tile([128, 512], F32, tag="ps_misc", name="psp_q")
                    for kt in range(2):
                        nc.tensor.matmul(
                            psp[:, 0:NQ],
                            lhsT=wq_bf[:, kt, mt * 128 : (mt + 1) * 128],
                            rhs=qT_bfs[b][:, kt, :],
                            start=(kt == 0),
                            stop=(kt == 1),
                        )
                    nc.vector.tensor_scalar(
                        out=qfT[:, mt, b], in0=psp[:, 0:NQ],
                        scalar1=CLAMP, scalar2=-CLAMP,
                        op0=ALU.min, op1=ALU.max,
                    )
            return qfT

        # ---------------- trig ladder ----------------
        def act_fundamentals(x, tl, act_squares=True):
            """ACT: s1, c1, s2 (and optionally their squares) of x."""
            t = {}
            for nm in ("s1", "c1", "s2", "s1sq", "s2sq"):
                t[nm] = tl(nm)
            nc.scalar.activation(out=t["s1"], in_=x, func=AF.Sin, scale=W0)
            nc.scalar.activation(out=t["c1"], in_=x, func=AF.Sin, scale=-W0,
                                 bias=halfpi[:, 0:1])
            nc.scalar.activation(out=t["s2"], in_=x, func=AF.Sin, scale=2 * W0)
            if act_squares:
                nc.scalar.activation(out=t["s1sq"], in_=t["s1"], func=AF.Square)
                nc.scalar.activation(out=t["s2sq"], in_=t["s2"], func=AF.Square)
            return t

        def dve_ladder(t, tl, dve_sq12=False, mid_cb=None):
            """DVE bf16 ladder; stored scales fold into BETA_F / TS imms."""
            TT, TS = nc.vector.tensor_tensor, nc.vector.tensor_scalar
            if dve_sq12:
                TT(out=t["s1sq"], in0=t["s1"], in1=t["s1"], op=ALU.mult)
                TT(out=t["s2sq"], in0=t["s2"], in1=t["s2"], op=ALU.mult)
            t["c2"] = tl("c2")
            TS(out=t["c2"], in0=t["s1sq"], scalar1=-2.0, scalar2=1.0,
               op0=ALU.mult, op1=ALU.add)
            # s3' = (s1sq - 3/4)*s1 = -sin3/4 ; c3' = (s1sq - 1/4)*c1 = -cos3/4
            # (the -1/4 factors fold into BETA_F and the TS immediates below)
            p3a = tl("p3a", tag="p3")
            TS(out=p3a, in0=t["s1sq"], scalar1=0.75, scalar2=None,
               op0=ALU.subtract)
            t["s3"] = tl("s3")
            TT(out=t["s3"], in0=t["s1"], in1=p3a, op=ALU.mult)
            p3b = tl("p3b", tag="p3")
            TS(out=p3b, in0=t["s1sq"], scalar1=0.25, scalar2=None,
               op0=ALU.subtract)
            t["c3"] = tl("c3")
            TT(out=t["c3"], in0=t["c1"], in1=p3b, op=ALU.mult)
            if mid_cb is not None:
                mid_cb()
            t["c4"] = tl("c4")
            TS(out=t["c4"], in0=t["s2sq"], scalar1=-2.0, scalar2=1.0,
               op0=ALU.mult, op1=ALU.add)
            t["s4"] = tl("s4")
            TT(out=t["s4"], in0=t["s2"], in1=t["c2"], op=ALU.mult)   # sin4/2
            ta, tb = tl("ta", tag="t0"), tl("tb", tag="t1")
            TT(out=ta, in0=t["s2"], in1=t["c3"], op=ALU.mult)
            TT(out=tb, in0=t["c2"], in1=t["s3"], op=ALU.mult)
            t["s5"] = tl("s5")
            TT(out=t["s5"], in0=ta, in1=tb, op=ALU.add)
            tc_, td = tl("tc", tag="t0"), tl("td", tag="t1")
            TT(out=tc_, in0=t["c2"], in1=t["c3"], op=ALU.mult)
            TT(out=td, in0=t["s2"], in1=t["s3"], op=ALU.mult)
            t["c5"] = tl("c5")
            TT(out=t["c5"], in0=tc_, in1=td, op=ALU.subtract)
            t["s6"] = tl("s6")
            TT(out=t["s6"], in0=t["s3"], in1=t["c3"], op=ALU.mult)   # sin6/2
            t["s8"] = tl("s8")
            TT(out=t["s8"], in0=t["s4"], in1=t["c4"], op=ALU.mult)   # sin8/4
            t["s10"] = tl("s10")
            TT(out=t["s10"], in0=t["s5"], in1=t["c5"], op=ALU.mult)  # sin10/2
            return t

        def ladder_highcos(t, tl, dve_squares=False):
            """squares of s3,s4,s5 (ACT or DVE) then DVE TS -> c6, c8, c10."""
            TS = nc.vector.tensor_scalar
            s3sq = tl("s3sq", tag="ssq")
            s4sq = tl("s4sq", tag="ssq2")
            s5sq = tl("s5sq", tag="ssq3")
            if dve_squares:
                nc.vector.tensor_tensor(out=s3sq, in0=t["s3"], in1=t["s3"],
                                        op=ALU.mult)
                nc.vector.tensor_tensor(out=s4sq, in0=t["s4"], in1=t["s4"],
                                        op=ALU.mult)
                nc.vector.tensor_tensor(out=s5sq, in0=t["s5"], in1=t["s5"],
                                        op=ALU.mult)
            else:
                nc.scalar.activation(out=s3sq, in_=t["s3"], func=AF.Square)
                nc.scalar.activation(out=s4sq, in_=t["s4"], func=AF.Square)
                nc.scalar.activation(out=s5sq, in_=t["s5"], func=AF.Square)
            t["c6"] = tl("c6")
            TS(out=t["c6"], in0=s3sq, scalar1=-32.0, scalar2=1.0,
               op0=ALU.mult, op1=ALU.add)
            t["c8"] = tl("c8")
            TS(out=t["c8"], in0=s4sq, scalar1=-8.0, scalar2=1.0,
               op0=ALU.mult, op1=ALU.add)
            t["c10"] = tl("c10")
            TS(out=t["c10"], in0=s5sq, scalar1=-32.0, scalar2=1.0,
               op0=ALU.mult, op1=ALU.add)
            return {m: (t[f"s{m}"], t[f"c{m}"]) for m in HSET}

        # ---------------- setup ----------------
        kfT0 = transpose_project_k(0)
        qfT = transpose_project_q()
        kfT1 = transpose_project_k(1)

        # mask bias rows [1, NK] (added into the score PSUM by a rank-1
        # matmul); ones row for the rank-1 lhsT
        ones_row = consts.tile([1, NQ], BF16)
        nc.gpsimd.memset(ones_row, 1.0)
        bias_bs = []
        for b in range(BPC):
            bias_b = setup.tile([1, NK], BF16, tag="bias", name=f"bias{b}")
            nc.vector.tensor_scalar(
                out=bias_b, in0=krow[0:1], scalar1=valid_fs[b][0:1, 0:1],
                scalar2=MASK_NEG, op0=ALU.is_ge, op1=ALU.mult,
            )
            bias_bs.append(bias_b)

        # tile factories: k-side standalone tiles; q-side sin/cos of each
        # harmonic share one tile so the wv-muls cover both in one op
        def tl_k(b):
            def tl(nm, tag=None):
                return lad.tile([128, 2, NK], BF16, tag=tag or nm,
                                name=f"k{b}_{nm}")
            return tl

        SCORE_NM = {}
        for m_ in HSET:
            SCORE_NM[f"s{m_}"] = (m_, 0)
            SCORE_NM[f"c{m_}"] = (m_, 1)
        qm = {m_: qlad.tile([128, 2, 2, BPC, NQ], BF16, tag=f"qm{m_}",
                            name=f"qm{m_}") for m_ in HSET}

        def tl_q(nm, tag=None):
            if nm in SCORE_NM:
                m_, fn_ = SCORE_NM[nm]
                return qm[m_][:, :, fn_]
            return qlad.tile([128, 2, BPC, NQ], BF16, tag=tag or ("q_" + nm),
                             name=f"q_{nm}")

        # ACT stream: q fundamentals first (q's ladder runs self-contained
        # on DVE so the lhs tensors unblock batch-0 scores early), then k0,
        # k1 fundamentals with their ACT squares
        qt_ = act_fundamentals(qfT, tl_q, act_squares=False)
        kt0 = act_fundamentals(kfT0, tl_k(0))
        kt1 = act_fundamentals(kfT1, tl_k(1))

        # q ladder first (all-DVE), then its wv-scaled lhs tensors
        dve_ladder(qt_, tl_q, dve_sq12=True)
        qtrig = ladder_highcos(qt_, tl_q, dve_squares=True)

        lhs = {}
        for i, m in enumerate(HSET):
            lt = qlad.tile([128, 2, 2, BPC, NQ], BF16, tag=f"lhs{m}",
                           name=f"lhs{m}")
            for ht in range(2):
                nc.vector.tensor_scalar(
                    out=lt[:, ht], in0=qm[m][:, ht],
                    scalar1=wv_col[:, ht : ht + 1], scalar2=BETA_F[i],
                    op0=ALU.mult, op1=ALU.mult,
                )
            lhs[m] = lt

        def cast_v(b):
            v_bf = outp.tile([128, 4, VD], BF16, tag="v_bf", name=f"v_bf{b}")
            for kb in range(4):
                nc.vector.tensor_copy(out=v_bf[:, kb], in_=v_sbs[b][:, kb])
            return v_bf

        # k0 ladder (highcos squares on ACT: they overlap the DVE stream)
        dve_ladder(kt0, tl_k(0))
        ktrig0 = ladder_highcos(kt0, tl_k(0))
        v_bfs = [cast_v(0), None]

        # ---------------- scores ----------------
        def score_matmuls(b, ktrig, sc_ps):
            # rank-1 matmul seeds the psum with the mask bias row
            nc.tensor.matmul(
                sc_ps[0:NQ], lhsT=ones_row, rhs=bias_bs[b],
                start=True, stop=False,
            )
            n = len(HSET)
            for i, m in enumerate(HSET):
                for fn in (0, 1):
                    # sin_q * cos_k  +  cos_q * sin_k
                    rhs = ktrig[m][1 - fn]
                    for ht in range(2):
                        nc.tensor.matmul(
                            sc_ps[0:NQ],
                            lhsT=lhs[m][:, ht, fn, b, :],
                            rhs=rhs[:, ht, :],
                            start=False,
                            stop=(i == n - 1 and fn == 1 and ht == 1),
                        )

        # ---------------- softmax + output ----------------
        def emit_exp(b, sc_ps):
            e_sb = sm.tile([NQ, NK], BF16, tag=f"e{b}", name=f"e{b}")
            denom = sm.tile([NQ, 1], F32, tag=f"den{b}", name=f"den{b}")
            # exp straight from the psum; normalization deferred to the
            # output copy (out rows scale by 1/denom there)
            nc.scalar.activation(out=e_sb, in_=sc_ps[0:NQ], func=AF.Exp,
                                 accum_out=denom)
            return e_sb, denom

        def emit_recip(b, denom):
            recip = sm.tile([NQ, 1], F32, tag=f"rec{b}", name=f"rec{b}")
            nc.vector.reciprocal(recip, denom)
            return recip

        def finish_out(b, attn, recip):
            attnT = outp.tile([128, 4, NQ], BF16, tag="attnT", name=f"attnT{b}")
            for kb in range(4):
                pst = ps_misc.tile([128, 1024], BF16, tag="ps_miscb", name="pst_a")
                nc.tensor.transpose(
                    pst[:, 0:NQ],
                    attn[:, kb * 128 : (kb + 1) * 128],
                    identb[0:NQ, 0:NQ],
                )
                nc.scalar.copy(out=attnT[:, kb], in_=pst[:, 0:NQ])

            po = ps_out.tile([NQ, VD], F32, tag="po", name=f"po{b}")
            for kb in range(4):
                nc.tensor.matmul(
                    po,
                    lhsT=attnT[:, kb],
                    rhs=v_bfs[b][:, kb],
                    start=(kb == 0),
                    stop=(kb == 3),
                )
            o_sb = outp.tile([NQ, VD], F32, tag="o_sb", name=f"o_sb{b}")
            nc.scalar.activation(out=o_sb, in_=po, func=AF.Identity,
                                 scale=recip[:, 0:1])
            nc.sync.dma_start(out=out_d[b][:, 0:128], in_=o_sb[:, 0:128])
            nc.gpsimd.dma_start(out=out_d[b][:, 128:256], in_=o_sb[:, 128:256])

        sc_ps0 = ps_sc.tile([128, NK], F32, tag="sc", name="sc0")
        score_matmuls(0, ktrig0, sc_ps0)
        e0, den0 = emit_exp(0, sc_ps0)
        rec0 = [None]

        # k1 ladder on DVE, with batch 0's reciprocal slotted into the
        # middle of the stream (right after exp0's accumulator lands)
        dve_ladder(kt1, tl_k(1), mid_cb=lambda: rec0.__setitem__(0, emit_recip(0, den0)))
        ktrig1 = ladder_highcos(kt1, tl_k(1), dve_squares=True)
        v_bfs[1] = cast_v(1)

        finish_out(0, e0, rec0[0])
        sc_ps1 = ps_sc.tile([128, NK], F32, tag="sc", name="sc1")
        score_matmuls(1, ktrig1, sc_ps1)
        e1, den1 = emit_exp(1, sc_ps1)
        rec1 = emit_recip(1, den1)
        finish_out(1, e1, rec1)

    nc.compile()
    return nc


_NC_CACHE = None
LAST_RESULTS = None


def kernel(queries, keys, values, valid_lens, W_q, W_k, w_v):
    global _NC_CACHE, LAST_RESULTS
    if _NC_CACHE is None:
        _NC_CACHE = _build()
    nc = _NC_CACHE

    queries = np.ascontiguousarray(queries, dtype=np.float32)
    keys = np.ascontiguousarray(keys, dtype=np.float32)
    values = np.ascontiguousarray(values, dtype=np.float32)
    valid_lens = np.ascontiguousarray(valid_lens, dtype=np.int32)
    W_q = np.ascontiguousarray(W_q, dtype=np.float32)
    W_k = np.ascontiguousarray(W_k, dtype=np.float32)
    w_v = np.ascontiguousarray(w_v, dtype=np.float32)

    in_maps = []
    for c in range(NCORES):
        lo, hi = c * BPC, (c + 1) * BPC
        in_maps.append(
            {
                "queries": queries[lo:hi],
                "keys": keys[lo:hi],
                "values": values[lo:hi],
                "valid_lens": valid_lens[lo:hi].reshape(BPC, 1),
                "W_q": W_q,
                "W_k": W_k,
                "w_v": w_v,
            }
        )

    trace = os.environ.get("ATTN_TRACE", "0") == "1"
    res = run_bass_kernel_spmd(
        nc, in_maps, core_ids=list(range(NCORES)), trace=trace
    )
    LAST_RESULTS = res
    return np.concatenate([r["out"] for r in res.results], axis=0)


# revision 30
# speedup vs baseline: 1.0775x; 1.0775x over previous
"""Additive attention (Bahdanau) Trainium2 kernel, 8-core data parallel.

out = softmax_k(mask(sum_h w_v[h] * tanh(qf[q,h] + kf[k,h]))) @ V
with qf = q@Wq, kf = k@Wk.

Instead of materializing tanh over the [nq, nk, H] feature tensor (16.8M
ACT elements per core -> ~110us floor), tanh is expanded in a sine series

    tanh(s) ~ sum_m beta_m sin(m*w0*s),   s = qf + kf

and each sin(m*w0*(a+b)) = sin(m*w0*a)cos(m*w0*b) + cos(m*w0*a)sin(m*w0*b)
is SEPARABLE: the score reduction over h becomes 2 matmuls per harmonic on
the (otherwise idle) PE.  Only the small projected tensors qf [64,256] and
kf [512,256] ever pass through trig:

  ACT:  sin/cos fundamentals (args kept in [-pi,pi] via a clamp fused into
        the PSUM evacuation) + squares, then the softmax exps
        (2 table loads total: trig_and_small, exp_and_others)
  DVE:  harmonic ladder in bf16 (double-angle + angle-addition products;
        scale factors from halved products folded into beta / TS imms)
  PE :  projections, 32 accumulating score matmuls per batch, attn @ V

Harmonics {1,2,3,4,5,6,8,10}, period 2P = 21 (w0 = 2pi/21), clamp 5.2.
Numpy-simulated end-to-end rel err vs the exact reference: 5.0e-3
(gate 2e-2).
"""

import os
from contextlib import ExitStack

import ml_dtypes
import numpy as np

import concourse.bacc as bacc
import concourse.bass as bass
import concourse.mybir as mybir
import concourse.tile as tile
from concourse.bass_utils import run_bass_kernel_spmd

F32 = mybir.dt.float32
BF16 = mybir.dt.bfloat16
I32 = mybir.dt.int32
AF = mybir.ActivationFunctionType
ALU = mybir.AluOpType

B, NQ, NK, QS, KS, H, VD = 16, 64, 512, 256, 256, 256, 256
NCORES = 8
BPC = B // NCORES  # batches per core
MASK_NEG = -30.0

PI = float(np.pi)
W0 = 2.0 * PI / 21.0       # fundamental frequency (P = 10.5)
CLAMP = 5.2                # |qf|,|kf| clamp so all ACT sin args stay in range
HSET = [1, 2, 3, 4, 5, 6, 8]
# ridge fit of tanh on [-10.5, 10.5], gaussian(sigma=sqrt(2)) + 1e-3 floor
# weighting, with the ladder's scale factors folded in: s3,c3,s5,c5 stored
# at -1/4 scale (x16 on 3,5), s4 at 1/2 (x2), s8 at 1/4 (x4), s6 at 1/32
BETA_F = [1.25311465, -0.03815384, 5.80332099, -0.15434926,
          3.12110416, -0.81435050, 0.22231956]


def _build():
    nc = bacc.Bacc()
    q_d = nc.declare_dram_parameter("queries", [BPC, NQ, QS], F32, isOutput=False)
    k_d = nc.declare_dram_parameter("keys", [BPC, NK, KS], F32, isOutput=False)
    v_d = nc.declare_dram_parameter("values", [BPC, NK, VD], F32, isOutput=False)
    vl_d = nc.declare_dram_parameter("valid_lens", [BPC, 1], I32, isOutput=False)
    wq_d = nc.declare_dram_parameter("W_q", [QS, H], F32, isOutput=False)
    wk_d = nc.declare_dram_parameter("W_k", [KS, H], F32, isOutput=False)
    wv_d = nc.declare_dram_parameter("w_v", [H], F32, isOutput=False)
    out_d = nc.declare_dram_parameter("out", [BPC, NQ, VD], F32, isOutput=True)

    identb_d = nc.inline_tensor(
        np.eye(128).astype(ml_dtypes.bfloat16), name="identb_c"
    )
    krow_d = nc.inline_tensor(np.arange(NK, dtype=np.float32), name="krow_c")

    with ExitStack() as ctx:
        tc = ctx.enter_context(tile.TileContext(nc))
        consts = ctx.enter_context(tc.tile_pool(name="consts", bufs=1))
        setup = ctx.enter_context(tc.tile_pool(name="setup", bufs=2))
        lad = ctx.enter_context(tc.tile_pool(name="lad", bufs=2))
        qlad = ctx.enter_context(tc.tile_pool(name="qlad", bufs=1))
        sm = ctx.enter_context(tc.tile_pool(name="sm", bufs=1))
        outp = ctx.enter_context(tc.tile_pool(name="outp", bufs=2))
        ps_sc = ctx.enter_context(tc.tile_pool(name="ps_sc", bufs=2, space="PSUM"))
        ps_misc = ctx.enter_context(tc.tile_pool(name="ps_misc", bufs=2, space="PSUM"))
        ps_out = ctx.enter_context(tc.tile_pool(name="ps_out", bufs=2, space="PSUM"))

        # ---------------- loads (critical first, two queues) ----------------
        # k blocks split by partition half so each lands on its own DMA
        # engine (per-engine throughput is the bottleneck, not queue count)
        k_sb0 = setup.tile([128, 4, KS], F32, tag="k_sb0", bufs=1)
        k0_view = k_d[0].rearrange("(kb p) d -> p kb d", p=128)
        identb = consts.tile([128, 128], BF16)
        for kb in range(4):
            nc.sync.dma_start(out=k_sb0[0:64, kb], in_=k0_view[0:64, kb])
            nc.gpsimd.dma_start(out=k_sb0[64:128, kb], in_=k0_view[64:128, kb])
        nc.gpsimd.dma_start(out=identb, in_=identb_d[:, :])
        wk_sb = setup.tile([128, 2, H], F32, tag="wk_f", bufs=1)
        nc.gpsimd.dma_start(out=wk_sb, in_=wk_d.rearrange("(kt p) m -> p kt m", p=128))
        wq_sb = setup.tile([128, 2, H], F32, tag="wq_f", bufs=1)
        nc.sync.dma_start(out=wq_sb, in_=wq_d.rearrange("(kt p) m -> p kt m", p=128))
        q_sb0 = setup.tile([NQ, QS], F32, tag="q_sb0", bufs=1)
        nc.sync.dma_start(out=q_sb0, in_=q_d[0])
        q_sb1 = setup.tile([NQ, QS], F32, tag="q_sb1", bufs=1)
        nc.sync.dma_start(out=q_sb1, in_=q_d[1])
        wv_col = consts.tile([128, 2], F32)
        nc.gpsimd.dma_start(out=wv_col, in_=wv_d.rearrange("(t p) -> p t", p=128))
        k_sb1 = setup.tile([128, 4, KS], F32, tag="k_sb1", bufs=1)
        k1_view = k_d[1].rearrange("(kb p) d -> p kb d", p=128)
        for kb in range(4):
            nc.sync.dma_start(out=k_sb1[0:64, kb], in_=k1_view[0:64, kb])
            nc.gpsimd.dma_start(out=k_sb1[64:128, kb], in_=k1_view[64:128, kb])
        krow = consts.tile([128, NK], F32)
        nc.sync.dma_start(out=krow, in_=krow_d[None, :].partition_broadcast(128))
        v_sbs, valid_sbs = [], []
        for b in range(BPC):
            v_sb = setup.tile([128, 4, VD], F32, tag=f"v_sb{b}", name=f"v_sb{b}", bufs=1)
            nc.gpsimd.dma_start(
                out=v_sb, in_=v_d[b].rearrange("(kb p) d -> p kb d", p=128)
            )
            v_sbs.append(v_sb)
            valid_sb = setup.tile([128, 1], I32, tag=f"valid{b}", name=f"valid{b}")
            nc.gpsimd.dma_start(
                out=valid_sb, in_=vl_d[b : b + 1, :].partition_broadcast(128)
            )
            valid_sbs.append(valid_sb)

        k_sbs = [k_sb0, k_sb1]
        q_sbs = [q_sb0, q_sb1]

        # ACT bias constants (per-partition APs)
        halfpi = consts.tile([128, 1], F32)
        nc.gpsimd.memset(halfpi, PI / 2)

        # weights to bf16 (first on the DVE stream); valid_lens to f32
        wq_bf = consts.tile([128, 2, H], BF16)
        wk_bf = consts.tile([128, 2, H], BF16)
        nc.vector.tensor_copy(out=wk_bf, in_=wk_sb)
        nc.vector.tensor_copy(out=wq_bf, in_=wq_sb)
        valid_fs = []
        for b in range(BPC):
            valid_f = setup.tile([128, 1], F32, tag="validf", name=f"vf{b}")
            nc.vector.tensor_copy(out=valid_f, in_=valid_sbs[b])
            valid_fs.append(valid_f)

        # ---------------- transposes + projections ----------------
        def transpose_project_k(b):
            """per-block bf16 cast -> bf16 PE transposes -> sliced projection."""
            k_bf = setup.tile([128, 4, KS], BF16, tag="k_bf", name=f"k_bf{b}")
            kT_bf = setup.tile([128, 2, NK], BF16, tag="kT", name=f"kT{b}")
            for kb in range(4):
                nc.vector.tensor_copy(out=k_bf[:, kb], in_=k_sbs[b][:, kb])
                for kt in range(2):
                    pst = ps_misc.tile([128, 1024], BF16, tag="ps_miscb", name="pst_k")
                    nc.tensor.transpose(
                        pst[:, 0:128], k_bf[:, kb, kt * 128 : (kt + 1) * 128], identb
                    )
                    nc.vector.tensor_copy(
                        out=kT_bf[:, kt, kb * 128 : (kb + 1) * 128], in_=pst[:, 0:128]
                    )
            kfT = setup.tile([128, 2, NK], F32, tag="kfT", name=f"kfT{b}")
            psps = []
            for mt in range(2):
                psps.append(ps_misc.tile([128, 512], F32, tag="ps_misc",
                                         name=f"psp_k{mt}"))
            # per-block slices so each projection only waits on its own block
            for mt in range(2):
                for kb in range(4):
                    sl = slice(kb * 128, (kb + 1) * 128)
                    for kt in range(2):
                        nc.tensor.matmul(
                            psps[mt][:, sl],
                            lhsT=wk_bf[:, kt, mt * 128 : (mt + 1) * 128],
                            rhs=kT_bf[:, kt, sl],
                            start=(kt == 0),
                            stop=(kt == 1),
                        )
            for mt in range(2):
                # evacuate + clamp to +-CLAMP in one DVE op
                nc.vector.tensor_scalar(
                    out=kfT[:, mt], in0=psps[mt], scalar1=CLAMP, scalar2=-CLAMP,
                    op0=ALU.min, op1=ALU.max,
                )
            return kfT

        def transpose_project_q():
            """Both batches' q -> combined clamped qfT [128, 2ht, 2b, NQ]."""
            qfT = qlad.tile([128, 2, BPC, NQ], F32, tag="qfT", name="qfT")
            qT_bfs = []
            for b in range(BPC):
                q_bf = setup.tile([NQ, QS], BF16, tag="q_bf", name=f"q_bf{b}")
                nc.vector.tensor_copy(out=q_bf, in_=q_sbs[b])
                qT_bf = setup.tile([128, 2, NQ], BF16, tag="qT", name=f"qT{b}")
                for kt in range(2):
                    pst = ps_misc.tile([128, 1024], BF16, tag="ps_miscb", name="pst_q")
                    nc.tensor.transpose(
                        pst[:, 0:NQ],
                        q_bf[:, kt * 128 : (kt + 1) * 128],
                        identb[0:NQ, 0:NQ],
                    )
                    nc.vector.tensor_copy(out=qT_bf[:, kt, :], in_=pst[:, 0:NQ])
                qT_bfs.append(qT_bf)
            for b in range(BPC):
                for mt in range(2):
                    psp = ps_misc.tile([128, 512], F32, tag="ps_misc", name="psp_q")
                    for kt in range(2):
                        nc.tensor.matmul(
                            psp[:, 0:NQ],
                            lhsT=wq_bf[:, kt, mt * 128 : (mt + 1) * 128],
                            rhs=qT_bfs[b][:, kt, :],
                            start=(kt == 0),
                            stop=(kt == 1),
                        )
                    nc.vector.tensor_scalar(
                        out=qfT[:, mt, b], in0=psp[:, 0:NQ],
                        scalar1=CLAMP, scalar2=-CLAMP,
                        op0=ALU.min, op1=ALU.max,
                    )
            return qfT

        # ---------------- trig ladder ----------------
        def act_fundamentals(x, tl, act_squares=True):
            """ACT: s1, c1, s2 (and optionally their squares) of x."""
            t = {}
            for nm in ("s1", "c1", "s2", "s1sq", "s2sq"):
                t[nm] = tl(nm)
            nc.scalar.activation(out=t["s1"], in_=x, func=AF.Sin, scale=W0)
            nc.scalar.activation(out=t["c1"], in_=x, func=AF.Sin, scale=-W0,
                                 bias=halfpi[:, 0:1])
            nc.scalar.activation(out=t["s2"], in_=x, func=AF.Sin, scale=2 * W0)
            if act_squares:
                nc.scalar.activation(out=t["s1sq"], in_=t["s1"], func=AF.Square)
                nc.scalar.activation(out=t["s2sq"], in_=t["s2"], func=AF.Square)
            return t

        def dve_ladder(t, tl, dve_sq12=False, mid_cb=None):
            """DVE bf16 ladder; stored scales fold into BETA_F / TS imms."""
            TT, TS = nc.vector.tensor_tensor, nc.vector.tensor_scalar
            if dve_sq12:
                TT(out=t["s1sq"], in0=t["s1"], in1=t["s1"], op=ALU.mult)
                TT(out=t["s2sq"], in0=t["s2"], in1=t["s2"], op=ALU.mult)
            t["c2"] = tl("c2")
            TS(out=t["c2"], in0=t["s1sq"], scalar1=-2.0, scalar2=1.0,
               op0=ALU.mult, op1=ALU.add)
            # s3' = (s1sq - 3/4)*s1 = -sin3/4 ; c3' = (s1sq - 1/4)*c1 = -cos3/4
            # (the -1/4 factors fold into BETA_F and the TS immediates below)
            p3a = tl("p3a", tag="p3")
            TS(out=p3a, in0=t["s1sq"], scalar1=0.75, scalar2=None,
               op0=ALU.subtract)
            t["s3"] = tl("s3")
            TT(out=t["s3"], in0=t["s1"], in1=p3a, op=ALU.mult)
            p3b = tl("p3b", tag="p3")
            TS(out=p3b, in0=t["s1sq"], scalar1=0.25, scalar2=None,
               op0=ALU.subtract)
            t["c3"] = tl("c3")
            TT(out=t["c3"], in0=t["c1"], in1=p3b, op=ALU.mult)
            if mid_cb is not None:
                mid_cb()
            t["c4"] = tl("c4")
            TS(out=t["c4"], in0=t["s2sq"], scalar1=-2.0, scalar2=1.0,
               op0=ALU.mult, op1=ALU.add)
            t["s4"] = tl("s4")
            TT(out=t["s4"], in0=t["s2"], in1=t["c2"], op=ALU.mult)   # sin4/2
            ta, tb = tl("ta", tag="t0"), tl("tb", tag="t1")
            TT(out=ta, in0=t["s2"], in1=t["c3"], op=ALU.mult)
            TT(out=tb, in0=t["c2"], in1=t["s3"], op=ALU.mult)
            t["s5"] = tl("s5")
            TT(out=t["s5"], in0=ta, in1=tb, op=ALU.add)
            tc_, td = tl("tc", tag="t0"), tl("td", tag="t1")
            TT(out=tc_, in0=t["c2"], in1=t["c3"], op=ALU.mult)
            TT(out=td, in0=t["s2"], in1=t["s3"], op=ALU.mult)
            t["c5"] = tl("c5")
            TT(out=t["c5"], in0=tc_, in1=td, op=ALU.subtract)
            t["s6"] = tl("s6")
            TT(out=t["s6"], in0=t["s3"], in1=t["c3"], op=ALU.mult)   # sin6/2
            t["s8"] = tl("s8")
            TT(out=t["s8"], in0=t["s4"], in1=t["c4"], op=ALU.mult)   # sin8/4
            return t

        def ladder_highcos(t, tl, dve_squares=False):
            """squares of s3,s4 (ACT or DVE) then DVE TS -> c6, c8."""
            TS = nc.vector.tensor_scalar
            s3sq = tl("s3sq", tag="ssq")
            s4sq = tl("s4sq", tag="ssq2")
            if dve_squares:
                nc.vector.tensor_tensor(out=s3sq, in0=t["s3"], in1=t["s3"],
                                        op=ALU.mult)
                nc.vector.tensor_tensor(out=s4sq, in0=t["s4"], in1=t["s4"],
                                        op=ALU.mult)
            else:
                nc.scalar.activation(out=s3sq, in_=t["s3"], func=AF.Square)
                nc.scalar.activation(out=s4sq, in_=t["s4"], func=AF.Square)
            t["c6"] = tl("c6")
            TS(out=t["c6"], in0=s3sq, scalar1=-32.0, scalar2=1.0,
               op0=ALU.mult, op1=ALU.add)
            t["c8"] = tl("c8")
            TS(out=t["c8"], in0=s4sq, scalar1=-8.0, scalar2=1.0,
               op0=ALU.mult, op1=ALU.add)
            return {m: (t[f"s{m}"], t[f"c{m}"]) for m in HSET}

        # ---------------- setup ----------------
        kfT0 = transpose_project_k(0)
        qfT = transpose_project_q()
        kfT1 = transpose_project_k(1)

        # mask bias rows [1, NK] (added into the score PSUM by a rank-1
        # matmul); ones row for the rank-1 lhsT
        ones_row = consts.tile([1, NQ], BF16)
        nc.gpsimd.memset(ones_row, 1.0)
        bias_bs = []
        for b in range(BPC):
            bias_b = setup.tile([1, NK], BF16, tag="bias", name=f"bias{b}")
            nc.vector.tensor_scalar(
                out=bias_b, in0=krow[0:1], scalar1=valid_fs[b][0:1, 0:1],
                scalar2=MASK_NEG, op0=ALU.is_ge, op1=ALU.mult,
            )
            bias_bs.append(bias_b)

        # tile factories: k-side standalone tiles; q-side sin/cos of each
        # harmonic share one tile so the wv-muls cover both in one op
        def tl_k(b):
            def tl(nm, tag=None):
                return lad.tile([128, 2, NK], BF16, tag=tag or nm,
                                name=f"k{b}_{nm}")
            return tl

        SCORE_NM = {}
        for m_ in HSET:
            SCORE_NM[f"s{m_}"] = (m_, 0)
            SCORE_NM[f"c{m_}"] = (m_, 1)
        qm = {m_: qlad.tile([128, 2, 2, BPC, NQ], BF16, tag=f"qm{m_}",
                            name=f"qm{m_}") for m_ in HSET}

        def tl_q(nm, tag=None):
            if nm in SCORE_NM:
                m_, fn_ = SCORE_NM[nm]
                return qm[m_][:, :, fn_]
            return qlad.tile([128, 2, BPC, NQ], BF16, tag=tag or ("q_" + nm),
                             name=f"q_{nm}")

        # ACT stream: k0 fundamentals first (batch 0's scores gate on the
        # k0 ladder tail), then q, then k1
        kt0 = act_fundamentals(kfT0, tl_k(0))
        qt_ = act_fundamentals(qfT, tl_q)
        kt1 = act_fundamentals(kfT1, tl_k(1))

        # DVE: k0 ladder first, then the q ladder + wv-scaled lhs tensors
        dve_ladder(kt0, tl_k(0))
        dve_ladder(qt_, tl_q)
        ktrig0 = ladder_highcos(kt0, tl_k(0))
        qtrig = ladder_highcos(qt_, tl_q)

        lhs = {}
        for i, m in enumerate(HSET):
            lt = qlad.tile([128, 2, 2, BPC, NQ], BF16, tag=f"lhs{m}",
                           name=f"lhs{m}")
            for ht in range(2):
                nc.vector.tensor_scalar(
                    out=lt[:, ht], in0=qm[m][:, ht],
                    scalar1=wv_col[:, ht : ht + 1], scalar2=BETA_F[i],
                    op0=ALU.mult, op1=ALU.mult,
                )
            lhs[m] = lt

        def cast_v(b):
            v_bf = outp.tile([128, 4, VD], BF16, tag="v_bf", name=f"v_bf{b}")
            for kb in range(4):
                nc.vector.tensor_copy(out=v_bf[:, kb], in_=v_sbs[b][:, kb])
            return v_bf

        v_bfs = [cast_v(0), None]

        # ---------------- scores ----------------
        def score_matmuls(b, ktrig, sc_ps):
            # rank-1 matmul seeds the psum with the mask bias row
            nc.tensor.matmul(
                sc_ps[0:NQ], lhsT=ones_row, rhs=bias_bs[b],
                start=True, stop=False,
            )
            n = len(HSET)
            for i, m in enumerate(HSET):
                for fn in (0, 1):
                    # sin_q * cos_k  +  cos_q * sin_k
                    rhs = ktrig[m][1 - fn]
                    for ht in range(2):
                        nc.tensor.matmul(
                            sc_ps[0:NQ],
                            lhsT=lhs[m][:, ht, fn, b, :],
                            rhs=rhs[:, ht, :],
                            start=False,
                            stop=(i == n - 1 and fn == 1 and ht == 1),
                        )

        # ---------------- softmax + output ----------------
        def emit_exp(b, sc_ps):
            e_sb = sm.tile([NQ, NK], BF16, tag=f"e{b}", name=f"e{b}")
            denom = sm.tile([NQ, 1], F32, tag=f"den{b}", name=f"den{b}")
            # exp straight from the psum; normalization deferred to the
            # output copy (out rows scale by 1/denom there)
            nc.scalar.activation(out=e_sb, in_=sc_ps[0:NQ], func=AF.Exp,
                                 accum_out=denom)
            return e_sb, denom

        def emit_recip(b, denom):
            recip = sm.tile([NQ, 1], F32, tag=f"rec{b}", name=f"rec{b}")
            nc.vector.reciprocal(recip, denom)
            return recip

        def finish_out(b, attn, recip):
            attnT = outp.tile([128, 4, NQ], BF16, tag="attnT", name=f"attnT{b}")
            for kb in range(4):
                pst = ps_misc.tile([128, 1024], BF16, tag="ps_miscb", name="pst_a")
                nc.tensor.transpose(
                    pst[:, 0:NQ],
                    attn[:, kb * 128 : (kb + 1) * 128],
                    identb[0:NQ, 0:NQ],
                )
                nc.scalar.copy(out=attnT[:, kb], in_=pst[:, 0:NQ])

            po = ps_out.tile([NQ, VD], F32, tag="po", name=f"po{b}")
            for kb in range(4):
                nc.tensor.matmul(
                    po,
                    lhsT=attnT[:, kb],
                    rhs=v_bfs[b][:, kb],
                    start=(kb == 0),
                    stop=(kb == 3),
                )
            o_sb = outp.tile([NQ, VD], F32, tag="o_sb", name=f"o_sb{b}")
            nc.scalar.activation(out=o_sb, in_=po, func=AF.Identity,
                                 scale=recip[:, 0:1])
            nc.sync.dma_start(out=out_d[b][:, 0:128], in_=o_sb[:, 0:128])
            nc.gpsimd.dma_start(out=out_d[b][:, 128:256], in_=o_sb[:, 128:256])

        sc_ps0 = ps_sc.tile([128, NK], F32, tag="sc", name="sc0")
        score_matmuls(0, ktrig0, sc_ps0)
        e0, den0 = emit_exp(0, sc_ps0)
        rec0 = [None]

        # k1 ladder on DVE, with batch 0's reciprocal slotted into the
        # middle of the stream (right after exp0's accumulator lands)
        dve_ladder(kt1, tl_k(1), mid_cb=lambda: rec0.__setitem__(0, emit_recip(0, den0)))
        ktrig1 = ladder_highcos(kt1, tl_k(1), dve_squares=True)
        v_bfs[1] = cast_v(1)

        finish_out(0, e0, rec0[0])
        sc_ps1 = ps_sc.tile([128, NK], F32, tag="sc", name="sc1")
        score_matmuls(1, ktrig1, sc_ps1)
        e1, den1 = emit_exp(1, sc_ps1)
        rec1 = emit_recip(1, den1)
        finish_out(1, e1, rec1)

    nc.compile()
    return nc


_NC_CACHE = None
LAST_RESULTS = None


def kernel(queries, keys, values, valid_lens, W_q, W_k, w_v):
    global _NC_CACHE, LAST_RESULTS
    if _NC_CACHE is None:
        _NC_CACHE = _build()
    nc = _NC_CACHE

    queries = np.ascontiguousarray(queries, dtype=np.float32)
    keys = np.ascontiguousarray(keys, dtype=np.float32)
    values = np.ascontiguousarray(values, dtype=np.float32)
    valid_lens = np.ascontiguousarray(valid_lens, dtype=np.int32)
    W_q = np.ascontiguousarray(W_q, dtype=np.float32)
    W_k = np.ascontiguousarray(W_k, dtype=np.float32)
    w_v = np.ascontiguousarray(w_v, dtype=np.float32)

    in_maps = []
    for c in range(NCORES):
        lo, hi = c * BPC, (c + 1) * BPC
        in_maps.append(
            {
                "queries": queries[lo:hi],
                "keys": keys[lo:hi],
                "values": values[lo:hi],
                "valid_lens": valid_lens[lo:hi].reshape(BPC, 1),
                "W_q": W_q,
                "W_k": W_k,
                "w_v": w_v,
            }
        )

    trace = os.environ.get("ATTN_TRACE", "0") == "1"
    res = run_bass_kernel_spmd(
        nc, in_maps, core_ids=list(range(NCORES)), trace=trace
    )
    LAST_RESULTS = res
    return np.concatenate([r["out"] for r in res.results], axis=0)
